# revision 1
# baseline (speedup 1.0000x reference)
"""CosineSimilarityLoss (histogram binning) Trainium2 kernel — radix-matmul.

Full inputs [2048, 4096] f32 x5 -> scalar f32 loss = 1 - mean(cosine_sim).

Strategy: data-parallel over 8 cores (256 rows each). Per row, the
2000-bin histogram dot-products are computed WITHOUT materializing the
histogram: bin = 64*h + s (radix split), and

  M[h', s'] = sum_k I_k * [h_k == h'] * [s_k == s']   (h' in [0,32), s' in [0,64))

is a tiny PE matmul  M = VI^T @ U  between one-hot factor matrices
  U [k, s']  = [s_k == s']        (64-wide)
  VI [k, h'] = I_k * [h_k == h']  (32-wide)
accumulated over 32 chunks of 128 items in PSUM.  Then
  dot = <M_P, M_T>,  pn2 = |M_P|^2,  tn2 = |M_T|^2   (per-row drains)
and cos = dot / sqrt(pn2*tn2).  Digits s,h are computed with a
round-to-nearest magic-add floor (exact except measure-zero ties), cast
to bf16 (exact small ints), transposed via PE so items lie on partitions,
and the one-hot builds run as bf16 tensor_tensor compares at DVE 2x rate
against broadcast digit columns.
"""

import os
import sys

sys.path.insert(0, "/opt/trn_rl_repo")

import numpy as np

import concourse.bass as bass
from concourse import bacc, mybir
from concourse.bass_utils import run_bass_kernel_spmd
from concourse.tile import TileContext
from concourse.masks import make_identity

N_CORES = int(os.environ.get("KV2_CORES", "8"))
B, P, T = 2048, 4096, 4096
RPC = 256  # rows per core
ROW_LIM = int(os.environ.get("KV2_ROWLIM", "128"))  # rows built per tile (debug)
STAGE = os.environ.get("KV2_STAGE", "full")  # mm | drain | full
DRAIN_MASK = int(os.environ.get("KV2_DRAIN", "7"))  # 1=copy 2=squares 4=ttred
NT = RPC // 128  # 2 row-tiles
NCH = P // 128  # 32 chunks per row
SW, HW_ = 64, 32  # radix widths: bin = 64*h + s
F32 = mybir.dt.float32
BF16 = mybir.dt.bfloat16
I32 = mybir.dt.int32
ALU = mybir.AluOpType
ACT = mybir.ActivationFunctionType
MAGIC = 12582912.0  # 1.5*2^23: keeps t+MAGIC in [2^23, 2^24) so ulp=1

_NC_CACHE = {}


def _bulk_digits(nc, y, ta, tb, dig_s, dig_h):
    """y (raw mz, f32, consumed as scratch) -> dig_s, dig_h (bf16).

    s = floor(y*2000) - 64*h, h = floor(y*2000/64), via rne(x - 0.5)
    magic-add sloppy floors (ping-pong through ta/tb, no in-place chains).
    """
    ts = nc.vector.tensor_scalar
    ts(out=ta[:], in0=y[:], scalar1=2000.0, scalar2=None,
       op0=ALU.mult)                               # ya = mz*2000
    ts(out=tb[:], in0=ta[:], scalar1=0.015625, scalar2=0.5,
       op0=ALU.mult, op1=ALU.subtract)             # t1 = ya/64 - 0.5
    ts(out=y[:], in0=tb[:], scalar1=MAGIC, scalar2=None,
       op0=ALU.add)                                # rounds at write
    ts(out=tb[:], in0=y[:], scalar1=MAGIC, scalar2=None,
       op0=ALU.subtract)                           # h (f32 integer)
    ts(out=y[:], in0=ta[:], scalar1=0.5, scalar2=MAGIC,
       op0=ALU.subtract, op1=ALU.add)              # rounds at write
    ts(out=ta[:], in0=y[:], scalar1=MAGIC, scalar2=None,
       op0=ALU.subtract)                           # fy = floor(ya)
    nc.vector.scalar_tensor_tensor(
        out=dig_s[:], in0=tb[:], scalar=-64.0, in1=ta[:],
        op0=ALU.mult, op1=ALU.add)                 # s = fy - 64h -> bf16
    ts(out=dig_h[:], in0=tb[:], scalar1=0.0, scalar2=None,
       op0=ALU.add)                                # cast h -> bf16


def build_nc():
    if "nc" in _NC_CACHE:
        return _NC_CACHE["nc"]
    nc = bacc.Bacc("TRN2", target_bir_lowering=False, debug=False,
                   num_devices=N_CORES)
    d_pmz = nc.dram_tensor("pmz", [RPC, P], F32, kind="ExternalInput")
    d_pint = nc.dram_tensor("pint", [RPC, P], F32, kind="ExternalInput")
    d_tmz = nc.dram_tensor("tmz", [RPC, T], F32, kind="ExternalInput")
    d_tint = nc.dram_tensor("tint", [RPC, T], F32, kind="ExternalInput")
    d_tmask = nc.dram_tensor("tmask", [RPC, T], F32, kind="ExternalInput")
    d_ident = nc.dram_tensor("ident", [128, 128], BF16, kind="ExternalInput")
    d_iotaU = nc.dram_tensor("iotaU", [128, SW * NCH], BF16,
                             kind="ExternalInput")
    d_iotaE = nc.dram_tensor("iotaE", [128, HW_ * NCH], BF16,
                             kind="ExternalInput")
    d_ones = nc.dram_tensor("ones", [HW_, 1], F32, kind="ExternalInput")
    d_cos = nc.dram_tensor("cos", [1, RPC], F32, kind="ExternalOutput")

    with TileContext(nc) as tc:
        with (
            tc.tile_pool(name="consts", bufs=1) as consts,
            tc.tile_pool(name="raw", bufs=1) as raw,
            tc.tile_pool(name="tmp", bufs=1) as tmp,
            tc.tile_pool(name="dig", bufs=1) as dig,
            tc.tile_pool(name="digT", bufs=1) as digT,
            tc.tile_pool(name="bld", bufs=2) as bld,
            tc.tile_pool(name="acc", bufs=1) as acc,
            tc.tile_pool(name="dr", bufs=2) as dr,
            tc.tile_pool(name="ptr", bufs=2, space="PSUM") as ptr,
            tc.tile_pool(name="pm", bufs=2, space="PSUM") as pm,
            tc.tile_pool(name="pfin", bufs=1, space="PSUM") as pfin,
        ):
            # ---- constants (DMA-ed from host; gpsimd is unreliable here) ----
            ident = consts.tile([128, 128], BF16, tag="ident")
            nc.sync.dma_start(ident[:], d_ident[:, :])
            iotaU = consts.tile([128, SW, NCH], BF16, tag="iotaU")
            nc.sync.dma_start(iotaU[:], d_iotaU[:, :])
            iotaE = consts.tile([128, HW_, NCH], BF16, tag="iotaE")
            nc.sync.dma_start(iotaE[:], d_iotaE[:, :])
            ones32 = consts.tile([HW_, 1], F32, tag="ones32")
            nc.sync.dma_start(ones32[:], d_ones[:, :])

            # ---- accumulators ----
            dot_all = acc.tile([HW_, RPC], F32, tag="dot_all")
            pn2_all = acc.tile([HW_, RPC], F32, tag="pn2_all")
            tn2_all = acc.tile([HW_, RPC], F32, tag="tn2_all")

            for t in range(NT):
                rs = slice(128 * t, 128 * (t + 1))
                # per-spectrum digit tiles in (r, c)-compact transposed form
                dTs = {}
                for sp, d_mz, d_int in (
                    ("p", d_pmz, d_pint),
                    ("t", d_tmz, d_tint),
                ):
                    y = raw.tile([128, P], F32, tag="raw_mz")
                    ii = raw.tile([128, P], F32, tag="raw_int")
                    nc.sync.dma_start(y[:], d_mz[rs, :])
                    nc.sync.dma_start(ii[:], d_int[rs, :])
                    ta = tmp.tile([128, P], F32, tag="ta")
                    tb = tmp.tile([128, P], F32, tag="tb")
                    dig_s = dig.tile([128, P], BF16, tag="dig_s")
                    dig_h = dig.tile([128, P], BF16, tag="dig_h")
                    dig_i = dig.tile([128, P], BF16, tag="dig_i")
                    _bulk_digits(nc, y, ta, tb, dig_s, dig_h)
                    if sp == "t":
                        mk = raw.tile([128, P], F32, tag="raw_mask")
                        nc.sync.dma_start(mk[:], d_tmask[rs, :])
                        nc.vector.scalar_tensor_tensor(
                            out=dig_i[:], in0=ii[:], scalar=0.0, in1=mk[:],
                            op0=ALU.bypass, op1=ALU.mult)
                    else:
                        nc.vector.tensor_scalar(
                            out=dig_i[:], in0=ii[:], scalar1=0.0,
                            scalar2=None, op0=ALU.add)
                    # transpose digits: [r, k] -> [k_lo, (r, c)] compact
                    sT = digT.tile([128, 128, NCH], BF16, tag=f"sT{sp}")
                    hT = digT.tile([128, 128, NCH], BF16, tag=f"hT{sp}")
                    iT = digT.tile([128, 128, NCH], BF16, tag=f"iT{sp}")
                    for src, dst in ((dig_s, sT), (dig_h, hT), (dig_i, iT)):
                        for g in range(NCH // 4):
                            pstg = ptr.tile([128, 4, 128], BF16, tag="pstg")
                            for b in range(4):
                                c = 4 * g + b
                                nc.tensor.transpose(
                                    pstg[:, b, :],
                                    src[:, 128 * c:128 * (c + 1)],
                                    ident[:])
                            nc.scalar.copy(
                                out=dst[:, :, 4 * g:4 * (g + 1)]
                                .transpose([0, 2, 1]),
                                in_=pstg[:])
                    dTs[sp] = (sT, hT, iT)

                for r in range(ROW_LIM):
                    rg = 128 * t + r
                    Ms = {}
                    for sp in ("p", "t"):
                        sT, hT, iT = dTs[sp]
                        U = bld.tile([128, SW, NCH], BF16, tag=f"U{sp}")
                        E = bld.tile([128, HW_, NCH], BF16, tag=f"E{sp}")
                        VI = bld.tile([128, HW_, NCH], BF16, tag=f"VI{sp}")
                        nc.vector.tensor_tensor(
                            out=U[:],
                            in0=sT[:, r:r + 1, :].to_broadcast(
                                [128, SW, NCH]),
                            in1=iotaU[:], op=ALU.is_equal)
                        nc.vector.tensor_tensor(
                            out=E[:],
                            in0=hT[:, r:r + 1, :].to_broadcast(
                                [128, HW_, NCH]),
                            in1=iotaE[:], op=ALU.is_equal)
                        nc.vector.tensor_tensor(
                            out=VI[:], in0=E[:],
                            in1=iT[:, r:r + 1, :].to_broadcast(
                                [128, HW_, NCH]),
                            op=ALU.mult)
                        if sp == "p":
                            M2 = pm.tile([HW_, 2, SW], F32, tag="M2")
                        M = M2[:, 0 if sp == "p" else 1, :]
                        for c in range(NCH):
                            nc.tensor.matmul(
                                M, VI[:, :, c], U[:, :, c],
                                start=(c == 0), stop=(c == NCH - 1))
                        Ms[sp] = M
                    MP, MT = Ms["p"], Ms["t"]
                    if STAGE == "mm":
                        continue
                    cp = dr.tile([HW_, SW], F32, tag="cp")
                    sq1 = dr.tile([HW_, SW], F32, tag="sq1")
                    sq2 = dr.tile([HW_, SW], F32, tag="sq2")
                    tts = dr.tile([HW_, SW], F32, tag="tts")
                    if DRAIN_MASK & 1:
                        nc.scalar.copy(out=cp[:], in_=MP)
                    else:
                        nc.vector.tensor_copy(out=cp[:], in_=MP)
                    if DRAIN_MASK & 2:
                        nc.scalar.activation(
                            sq1[:], MP, ACT.Square,
                            accum_out=pn2_all[:, rg:rg + 1])
                        nc.scalar.activation(
                            sq2[:], MT, ACT.Square,
                            accum_out=tn2_all[:, rg:rg + 1])
                    if DRAIN_MASK & 4:
                        nc.vector.scalar_tensor_tensor(
                            out=tts[:], in0=cp[:], scalar=0.0, in1=MT,
                            op0=ALU.bypass, op1=ALU.mult,
                            accum_out=dot_all[:, rg:rg + 1])

            # ---- finale: cross-partition reduce + cosine tail ----
            if STAGE in ("mm", "drain"):
                cz = acc.tile([1, RPC], F32, tag="cz")
                if STAGE == "drain":
                    nc.scalar.copy(out=cz[:], in_=dot_all[0:1, :])
                else:
                    nc.vector.memset(cz[:], 0.0)
                nc.sync.dma_start(d_cos[:], cz[:])
                raise_skip = True
            else:
                raise_skip = False
            if not raise_skip:
                fin3 = pfin.tile([1, 3, RPC], F32, tag="fin3")
                fdot, fpn, ftn = (fin3[:, 0, :], fin3[:, 1, :],
                                  fin3[:, 2, :])
                nc.tensor.matmul(fdot, ones32[:], dot_all[:],
                                 start=True, stop=True)
                nc.tensor.matmul(fpn, ones32[:], pn2_all[:],
                                 start=True, stop=True)
                nc.tensor.matmul(ftn, ones32[:], tn2_all[:],
                                 start=True, stop=True)
                dn = acc.tile([1, RPC], F32, tag="dn")
                pn = acc.tile([1, RPC], F32, tag="pn")
                tn = acc.tile([1, RPC], F32, tag="tn")
                den = acc.tile([1, RPC], F32, tag="den")
                cosv = acc.tile([1, RPC], F32, tag="cosv")
                nc.scalar.copy(out=dn[:], in_=fdot)
                nc.scalar.activation(pn[:], fpn, ACT.Sqrt)
                nc.scalar.activation(tn[:], ftn, ACT.Sqrt)
                rden = acc.tile([1, RPC], F32, tag="rden")
                nc.vector.tensor_tensor(out=den[:], in0=pn[:], in1=tn[:],
                                        op=ALU.mult)
                nc.vector.reciprocal(rden[:], den[:])
                nc.vector.tensor_tensor(out=cosv[:], in0=dn[:],
                                        in1=rden[:], op=ALU.mult)
                nc.sync.dma_start(d_cos[:], cosv[:])
    nc.compile()
    _NC_CACHE["nc"] = nc
    return nc


def _consts():
    import ml_dtypes

    ident = np.eye(128, dtype=ml_dtypes.bfloat16)
    iotaU = np.broadcast_to(
        np.arange(SW, dtype=np.float32)[:, None], (SW, NCH)
    ).reshape(1, -1).repeat(128, 0).astype(ml_dtypes.bfloat16)
    iotaE = np.broadcast_to(
        np.arange(HW_, dtype=np.float32)[:, None], (HW_, NCH)
    ).reshape(1, -1).repeat(128, 0).astype(ml_dtypes.bfloat16)
    ones = np.ones((HW_, 1), dtype=np.float32)
    return {"ident": ident, "iotaU": iotaU, "iotaE": iotaE, "ones": ones}


def make_in_maps(np_inputs):
    cst = _consts()
    in_maps = []
    for c in range(N_CORES):
        rs = slice(c * RPC, (c + 1) * RPC)
        in_maps.append(
            {
                "pmz": np.ascontiguousarray(np_inputs["pred_mz"][rs]),
                "pint": np.ascontiguousarray(np_inputs["pred_intensity"][rs]),
                "tmz": np.ascontiguousarray(np_inputs["target_mz"][rs]),
                "tint": np.ascontiguousarray(np_inputs["target_intensity"][rs]),
                "tmask": np.ascontiguousarray(np_inputs["target_mask"][rs]),
                **cst,
            }
        )
    return in_maps


def kernel(pred_mz, pred_intensity, target_mz, target_intensity, target_mask):
    np_inputs = {
        "pred_mz": np.ascontiguousarray(pred_mz, dtype=np.float32),
        "pred_intensity": np.ascontiguousarray(pred_intensity, dtype=np.float32),
        "target_mz": np.ascontiguousarray(target_mz, dtype=np.float32),
        "target_intensity": np.ascontiguousarray(target_intensity, dtype=np.float32),
        "target_mask": np.ascontiguousarray(target_mask, dtype=np.float32),
    }
    nc = build_nc()
    in_maps = make_in_maps(np_inputs)
    res = run_bass_kernel_spmd(nc, in_maps, core_ids=list(range(N_CORES)))
    cos = np.concatenate([r["cos"].reshape(-1) for r in res.results])
    mean = np.mean(cos.astype(np.float64))
    return np.float32(1.0 - mean)

